# revision 1
# baseline (speedup 1.0000x reference)
"""Trainium2 Bass kernel for the scan-RNN problem (B=2048, T=512, H=256).

Data-parallel over batch: 8 cores x 256 rows each. The T=512 recurrence runs
fully on-chip per core; weights are replicated.

Math (per step, after host-side folding of gamma/beta into W_update/W_out):
    z   = (h + tanh(x_t*W_embed + b_embed)) @ W' + b'
    u   = tanh(z)
    h   = (u - mean(u)) * rsqrt(var(u) + eps)
Since x values are integers 0..9, tanh(x*W_embed+b_embed) is a 10-row table E;
inp @ W' = onehot(x) @ (E @ W'), so the embedding path becomes a K=11 matmul
with a host-precomputed one-hot (row 10 = ones carries the bias b').

On-chip layout per core: state is kept transposed (hT[j, b]) so each step's
matmul needs no extra transpose on the input side; the step output is
re-transposed with PE transpose-mode matmuls.

All constants live in one host-packed blob loaded by a single DMA so no
instruction accumulates more semaphore waits than the ISA allows.
"""

import numpy as np

H = 256
EPS = 1e-5
NCORES = 8
NV = 10  # x values are 0..9
KAUG = NV + 1  # + ones row for the bias
OHB = 16  # one-hot steps per DMA batch

MAGIC = 0x5F3759DF  # rsqrt seed magic (int32 trick done via f32 converts)

# blob column layout (all fp32, 128 partitions)
_WP0 = 0          # W' chunk 0 lhsT [128, 256]
_WP1 = 256        # W' chunk 1 lhsT [128, 256]
_ID = 512         # identity [128, 128]
_GA = 640         # G_aug [11(part), 256]
_WO = 896         # W_out' chunks [128, 2, 16]
_BO = 928         # row 0: b_out' [1, 16]; row 1..: zeros
_ONES = 944       # ones [1, 128] on partition 0
_ZERO = 1072      # zeros [128, 512] = h0 transposed state
_CW = 1072 + 512  # blob width


def build_nc(T, B_local):
    """Build the Bass program for one core (SPMD: all cores identical)."""
    import concourse.bass as bass
    import concourse.mybir as mybir
    import concourse.tile as tile
    from concourse import bacc

    dt = mybir.dt
    AF = mybir.ActivationFunctionType
    nc = bacc.Bacc(None, target_bir_lowering=False, debug=False)

    NB = B_local // 128  # batch half-tiles (2)
    assert B_local % 128 == 0

    # ---- DRAM parameters -------------------------------------------------
    assert T % OHB == 0 or T < OHB
    ohb = min(OHB, T)
    oh = nc.declare_dram_parameter(
        "oh", [(T + ohb - 1) // ohb, KAUG, ohb * B_local], dt.float32,
        isOutput=False)
    cst = nc.declare_dram_parameter("cst", [128, _CW], dt.float32,
                                    isOutput=False)
    out = nc.declare_dram_parameter("out", [B_local, 16], dt.float32,
                                    isOutput=True)

    with tile.TileContext(nc) as tc:
        with (
            tc.tile_pool(name="singles", bufs=1) as singles,
            tc.tile_pool(name="ohpool", bufs=8) as ohpool,
            tc.tile_pool(name="state", bufs=2) as state,
            tc.tile_pool(name="work", bufs=2) as work,
            tc.tile_pool(name="stats", bufs=2) as stats,
            tc.tile_pool(name="psum_z", bufs=2, space="PSUM") as psum_z,
            tc.tile_pool(name="psum_t", bufs=1, space="PSUM") as psum_t,
            tc.tile_pool(name="psum_j", bufs=1, space="PSUM") as psum_j,
        ):
            # ---- one DMA for every constant -----------------------------
            blob = singles.tile([128, _CW], dt.float32, tag="blob")
            nc.sync.dma_start(out=blob, in_=cst[:, :])
            wp0 = blob[:, _WP0:_WP0 + H]
            wp1 = blob[:, _WP1:_WP1 + H]
            ident = blob[:, _ID:_ID + 128]
            ga_sb = blob[:KAUG, _GA:_GA + H]
            wo_sb = blob[:, _WO:_WO + 32].rearrange("p (c h) -> p c h", c=2)
            bo_sb = blob[:1, _BO:_BO + 16]
            ones_row = blob[:1, _ONES:_ONES + 128]
            h0 = blob[:, _ZERO:_ZERO + 2 * B_local].rearrange(
                "p (c b) -> p c b", c=2)

            # per-half state tiles: hT[h][q, c, b] with b in [0,128)
            hTs = [h0[:, :, 0:128], h0[:, :, 128:256]]
            oh_bt = None
            for t in range(T):
                # ---- batched one-hot load (one DMA per OHB steps) -------
                if t % ohb == 0:
                    oh_bt = ohpool.tile([KAUG, ohb, B_local], dt.float32,
                                        tag="oh")
                    nc.sync.dma_start(
                        out=oh_bt,
                        in_=oh[t // ohb, :, :].rearrange(
                            "v (s b) -> v s b", s=ohb),
                    )

                # ---- per-half independent pipelines ---------------------
                pzs, us, scrs = [], [], []
                sums = stats.tile([128, 2, NB], dt.float32, tag="sums")
                usum = sums[:, 0, :]   # [128, NB]
                sqsum = sums[:, 1, :]
                for hb in range(NB):
                    bs = bass.ts(hb, 128)
                    pz = psum_z.tile([128, H], dt.float32, tag=f"pz{hb}")
                    # G matmul first: depends only on the one-hot DMA
                    nc.tensor.matmul(
                        pz, lhsT=oh_bt[:, t % ohb, bs], rhs=ga_sb,
                        start=True, stop=False,
                    )
                    nc.tensor.matmul(
                        pz, lhsT=hTs[hb][:, 0, :], rhs=wp0,
                        start=False, stop=False,
                    )
                    nc.tensor.matmul(
                        pz, lhsT=hTs[hb][:, 1, :], rhs=wp1,
                        start=False, stop=True,
                    )
                    # tanh evacuates PSUM, accumulates the row-sum (mean)
                    u = work.tile([128, H], dt.float32, tag=f"u{hb}")
                    nc.scalar.activation(
                        u, pz, AF.Tanh, accum_out=usum[:, hb:hb + 1],
                    )
                    pzs.append(pz)
                    us.append(u)

                # sumsq: one fused mul + one reduce across both halves
                scr = work.tile([128, NB, H], dt.float32, tag="scr")
                for hb in range(NB):
                    nc.vector.tensor_mul(scr[:, hb, :], us[hb], us[hb])
                nc.vector.tensor_reduce(
                    out=sums[:, 1, :], in_=scr,
                    axis=mybir.AxisListType.X, op=mybir.AluOpType.add,
                )

                # PE warm-keepers: tiny matmuls dependent on mid-chain data
                junk = psum_j.tile([16, 16], dt.float32, tag="junk")
                nc.tensor.matmul(junk, lhsT=us[0][:, 0:16],
                                 rhs=ident[:, 0:16], start=True, stop=True)
                nc.tensor.matmul(junk, lhsT=scr[:, 0, 0:16],
                                 rhs=ident[:, 0:16], start=True, stop=True)

                # ---- shared stats chain on [128, NB] columns ------------
                sc = stats.tile([128, 5, NB], dt.float32, tag="sc")
                mean = sc[:, 0, :]
                ve = sc[:, 1, :]
                y0 = sc[:, 2, :]
                tt = sc[:, 3, :]
                y1 = sc[:, 4, :]
                nc.vector.tensor_scalar_mul(mean, usum, 1.0 / H)
                # tt = mean^2 - eps ; ve = sqsum/H - tt
                nc.vector.tensor_mul(tt, mean, mean)
                nc.vector.tensor_scalar_add(tt, tt, -EPS)
                nc.vector.scalar_tensor_tensor(
                    out=ve, in0=sqsum, scalar=1.0 / H, in1=tt,
                    op0=mybir.AluOpType.mult, op1=mybir.AluOpType.subtract,
                )
                # seed: float(bits(ve)) -> linear -> int -> bits as float
                nc.vector.tensor_copy(out=y0, in_=ve.bitcast(dt.int32))
                nc.vector.tensor_scalar(
                    out=y0, in0=y0, scalar1=-0.5, scalar2=float(MAGIC),
                    op0=mybir.AluOpType.mult, op1=mybir.AluOpType.add,
                )
                nc.vector.tensor_copy(out=y0.bitcast(dt.int32), in_=y0)
                # Newton x2 on [128, NB] via tensor-tensor ops
                for ycur, ynext in ((y0, y1), (y1, ve)):
                    nc.vector.tensor_mul(tt, ycur, ycur)
                    nc.vector.tensor_mul(tt, tt, ve)
                    nc.vector.tensor_scalar(
                        out=tt, in0=tt, scalar1=-0.5, scalar2=1.5,
                        op0=mybir.AluOpType.mult, op1=mybir.AluOpType.add,
                    )
                    nc.vector.tensor_mul(ynext, tt, ycur)
                rstd = ve  # [128, NB]

                # ---- apply + transpose + copy, per half -----------------
                new_hTs = []
                for hb in range(NB):
                    hn = work.tile([128, H], dt.float32, tag=f"hn{hb}")
                    nc.vector.tensor_scalar(
                        out=hn, in0=us[hb],
                        scalar1=mean[:, hb:hb + 1], scalar2=rstd[:, hb:hb + 1],
                        op0=mybir.AluOpType.subtract, op1=mybir.AluOpType.mult,
                    )
                    pt = psum_t.tile([128, 2, 128], dt.float32, tag=f"pt{hb}")
                    for c in range(2):
                        # transpose as a plain matmul: out = hn_blk.T @ I
                        nc.tensor.matmul(
                            pt[:, c, :], lhsT=hn[:, bass.ts(c, 128)],
                            rhs=ident, start=True, stop=True,
                        )
                    hT = state.tile([128, 2, 128], dt.float32, tag=f"hT{hb}")
                    # split the PSUM evacuation across ACT and DVE
                    nc.scalar.copy(out=hT[:, 0, :], in_=pt[:, 0, :])
                    nc.vector.tensor_copy(out=hT[:, 1, :], in_=pt[:, 1, :])
                    new_hTs.append(hT)
                hTs = new_hTs

            # ---- final projection: out = h @ Wout' + bout' --------------
            po = psum_t.tile([128, NB, 16], dt.float32, tag="po")
            for hb in range(NB):
                nc.tensor.matmul(
                    po[:, hb, :], lhsT=hTs[hb][:, 0, :], rhs=wo_sb[:, 0, :],
                    start=True, stop=False,
                )
                nc.tensor.matmul(
                    po[:, hb, :], lhsT=hTs[hb][:, 1, :], rhs=wo_sb[:, 1, :],
                    start=False, stop=False,
                )
                nc.tensor.matmul(
                    po[:, hb, :], lhsT=ones_row, rhs=bo_sb,
                    start=False, stop=True,
                )
            ot = work.tile([128, NB, 16], dt.float32, tag="ot")
            nc.vector.tensor_copy(out=ot, in_=po)
            nc.sync.dma_start(
                out=out[:, :].rearrange("(c p) h -> p c h", p=128), in_=ot
            )

    nc.finalize()
    return nc


def _prepare_host(x, W_embed, b_embed, W_update, b_update, gamma, beta,
                  W_out, b_out):
    """Fold gamma/beta into the weights; build one-hot + the consts blob."""
    Wp = (gamma[:, None] * W_update).astype(np.float32)  # [H, H]
    bp = (b_update + beta @ W_update).astype(np.float32)  # [H]
    Wo = (gamma[:, None] * W_out).astype(np.float32)  # [H, 10]
    bo = (b_out + beta @ W_out).astype(np.float32)  # [10]

    vals = np.arange(NV, dtype=np.float32)[:, None]
    E = np.tanh(vals @ W_embed + b_embed).astype(np.float32)  # [10, H]
    G = (E @ Wp).astype(np.float32)
    G_aug = np.concatenate([G, bp[None, :]], axis=0)  # [KAUG, H]

    xi = x[:, :, 0].astype(np.int32)  # [B, T]
    B, T = xi.shape
    oh = np.zeros((T, KAUG, B), np.float32)
    tidx = np.broadcast_to(np.arange(T)[:, None], (T, B))
    bidx = np.broadcast_to(np.arange(B)[None, :], (T, B))
    oh[tidx, xi.T, bidx] = 1.0
    oh[:, NV, :] = 1.0

    cst = np.zeros((128, _CW), np.float32)
    cst[:, _WP0:_WP0 + H] = Wp[0:128]
    cst[:, _WP1:_WP1 + H] = Wp[128:256]
    cst[:, _ID:_ID + 128] = np.eye(128, dtype=np.float32)
    cst[:KAUG, _GA:_GA + H] = G_aug
    cst[:, _WO:_WO + 16] = np.pad(Wo[0:128], ((0, 0), (0, 6)))
    cst[:, _WO + 16:_WO + 32] = np.pad(Wo[128:256], ((0, 0), (0, 6)))
    cst[0, _BO:_BO + 10] = bo
    cst[0, _ONES:_ONES + 128] = 1.0
    # _ZERO region stays zero = h0
    return oh, cst


def prepare(x, W_embed, b_embed, W_update, b_update, gamma, beta, W_out, b_out,
            T_override=None, B_override=None):
    x = np.asarray(x, np.float32)
    B = x.shape[0] if B_override is None else B_override
    T = x.shape[1] if T_override is None else T_override
    x = x[:B, :T]

    oh, cst = _prepare_host(
        np.asarray(x), np.asarray(W_embed), np.asarray(b_embed),
        np.asarray(W_update), np.asarray(b_update), np.asarray(gamma),
        np.asarray(beta), np.asarray(W_out), np.asarray(b_out),
    )

    B_local = B // NCORES
    nc = build_nc(T, B_local)

    ohb = min(OHB, T)
    in_maps = []
    for c in range(NCORES):
        sl = slice(c * B_local, (c + 1) * B_local)
        ohc = oh[:, :, sl]  # [T, KAUG, B_local]
        ohc = ohc.reshape(T // ohb, ohb, KAUG, B_local).transpose(0, 2, 1, 3)
        ohc = ohc.reshape(T // ohb, KAUG, ohb * B_local)
        in_maps.append({
            "oh": np.ascontiguousarray(ohc),
            "cst": cst,
        })
    return nc, in_maps


def _numpy_fallback(x, W_embed, b_embed, W_update, b_update, gamma, beta,
                    W_out, b_out):
    """Reference math on host; only for inputs the device kernel can't take
    (non-integer x or values outside 0..9 - never happens with the spec'd
    randint fill, but better safe than crashed)."""
    xb = x[:, :, 0]
    B, T = xb.shape
    h = np.zeros((B, H), np.float32)
    for t in range(T):
        inp = np.tanh(xb[:, t:t + 1] @ W_embed + b_embed)
        z = (inp + h) @ W_update + b_update
        u = np.tanh(z)
        mu = u.mean(-1, keepdims=True)
        var = ((u - mu) ** 2).mean(-1, keepdims=True)
        h = (u - mu) / np.sqrt(var + EPS) * gamma + beta
    return (h @ W_out + b_out).astype(np.float32)


def kernel(x, W_embed, b_embed, W_update, b_update, gamma, beta, W_out, b_out,
           T_override=None, B_override=None):
    x = np.asarray(x, np.float32)
    xi = x[:, :, 0]
    if not (np.all(xi == np.round(xi)) and xi.min() >= 0 and xi.max() < NV
            and x.shape[0] % (NCORES * 128) == 0):
        return _numpy_fallback(
            x, np.asarray(W_embed, np.float32), np.asarray(b_embed, np.float32),
            np.asarray(W_update, np.float32), np.asarray(b_update, np.float32),
            np.asarray(gamma, np.float32), np.asarray(beta, np.float32),
            np.asarray(W_out, np.float32), np.asarray(b_out, np.float32))

    nc, in_maps = prepare(x, W_embed, b_embed, W_update, b_update, gamma, beta,
                          W_out, b_out, T_override, B_override)

    from concourse.bass_utils import run_bass_kernel_spmd

    res = run_bass_kernel_spmd(nc, in_maps, list(range(NCORES)))
    global LAST_RESULT
    LAST_RESULT = res
    outs = [res.results[c]["out"][:, :10] for c in range(NCORES)]
    return np.concatenate(outs, axis=0).astype(np.float32)


LAST_RESULT = None

